# revision 3
# baseline (speedup 1.0000x reference)
"""AWQ int4 matmul kernel for Trainium2 (8 NeuronCores, tensor-parallel on out dim).

Computes: out[b,s,o] = sum_k (input[b,s,k]/eq_scales[k]) * ((int_weight-zeros)*scales)[o,k] + bias[o]

Strategy v7: the device runs a PURE bf16 GEMM; all AWQ marshalling is host-side.
  - Host ships, per core: xt[p, kc, t] = (x/eq)[t, kc*128+p] bf16 (K-major,
    token-contiguous, replicated); wt = dequantized W^T per 128-out chunk
    ((w-z)*s in f32, one bf16 rounding, zero-padded to 11 chunks, packed so
    each chunk is one contiguous 1 MB DMA); bt[p, oc] = bias.
  - Device, per out-chunk: DMA the wt chunk (ACT ring, prefetch depth 4) ->
    32x4 N=512 matmuls (stationary wT[:,kc,:], moving 512-token x slices,
    4-PSUM-bank rotation) -> per-partition bias add (DVE) -> bf16 out^T
    store (SWDGE). 1408 MMs/core; LDWEIGHTS is hidden (~3ns), measured
    ~250 ns/MM stream rate.
  - Warm-up A-phase: first 4 chunks run token-slice 0 only while x streams,
    chunk-PAIRED (consecutive MMs alternate banks+stationaries: same-bank
    back-to-back MMs cost ~15ns). B-phase: their remaining slices. The LAST
    chunk retires slices progressively ([0],[1],[2,3]) so the next
    iteration's x loads start staggered before the iteration boundary.
  - Output computed TRANSPOSED (out^T [1376,2048] bf16): out-features on
    PSUM partitions make the bias a per-partition tensor_scalar; host
    transposes/upcasts.
cfg ablation switches (timing experiments only): no_store, hoist_x, hoist_w,
n_a (warm-up chunk count).
"""

import sys

sys.path.insert(0, "/opt/trn_rl_repo")

from contextlib import ExitStack

import numpy as np
import ml_dtypes

import concourse.bass as bass
import concourse.mybir as mybir
import concourse.tile as tile
from concourse import bacc
from concourse.bass_utils import run_bass_kernel_spmd

dt = mybir.dt

OUT, IN, GROUP = 11008, 4096, 128
NG = IN // GROUP
B, S = 2, 1024
T = B * S
N_CORES = 8
O_PC = OUT // N_CORES
TS = 512
P_HOST = 128


def build_body(ctx, tc, cfg):
    nc = tc.nc
    P = 128
    T_, IN_, O_ = cfg["t"], cfg["in_"], cfg["o_pc"]
    NG_ = IN_ // GROUP
    n_ts = T_ // TS
    n_oc = (O_ + P - 1) // P
    no_store = cfg.get("no_store", 0)
    hoist_x = cfg.get("hoist_x", 0)
    hoist_w = cfg.get("hoist_w", 0)
    n_a = min(cfg.get("n_a", 4), n_oc)
    w_bufs = max(n_a + 1, 5)

    x_d = nc.dram_tensor("xt", [P, NG_ * T_], dt.bfloat16, kind="ExternalInput").ap()
    w_d = nc.dram_tensor(
        "wt", [n_oc * P, NG_ * P], dt.bfloat16, kind="ExternalInput"
    ).ap()
    bt_d = nc.dram_tensor("bt", [P, n_oc], dt.float32, kind="ExternalInput").ap()
    out_d = nc.dram_tensor("out", [O_, T_], dt.bfloat16, kind="ExternalOutput").ap()

    consts = ctx.enter_context(tc.tile_pool(name="consts", bufs=1))
    xrpool = ctx.enter_context(tc.tile_pool(name="xres", bufs=1))
    wTp = ctx.enter_context(tc.tile_pool(name="wT", bufs=w_bufs))
    pspool = ctx.enter_context(tc.tile_pool(name="ps", bufs=8, space="PSUM"))
    opool = ctx.enter_context(tc.tile_pool(name="osb", bufs=3))

    xres = xrpool.tile([P, NG_, T_], dt.bfloat16, tag="xres")

    def x_load(ts):
        src = bass.AP(
            tensor=x_d.tensor,
            offset=x_d.offset + ts * TS,
            ap=[[NG_ * T_, P], [T_, NG_], [1, TS]],
        )
        nc.sync.dma_start(xres[:, :, ts * TS : (ts + 1) * TS], src)

    def w_load(oc):
        wT = wTp.tile([P, NG_, P], dt.bfloat16, tag="wT", name="wT")
        src2 = w_d[oc * P : (oc + 1) * P, :]
        src = bass.AP(
            tensor=src2.tensor,
            offset=src2.offset,
            ap=[list(src2.ap[0]), [P, NG_], [1, P]],
        )
        nc.scalar.dma_start(wT[:], src)
        return wT

    b_sb = consts.tile([P, n_oc], dt.float32, tag="b_sb")

    # hoisted (timing-ablation) loads
    pre_wTs = {}
    if hoist_x:
        for ts in range(n_ts):
            x_load(ts)
    if hoist_w:
        # timing-only: w DMA removed from the loop; chunks >= w_bufs reuse
        # slots (wrong data, same MM pattern)
        for oc in range(w_bufs):
            pre_wTs[oc] = w_load(oc)
        for oc in range(w_bufs, n_oc):
            pre_wTs[oc] = pre_wTs[oc % w_bufs]

    def emit_iter():
        nc.gpsimd.dma_start(b_sb[:], bt_d)

        def store(oc, ts, ps):
            if no_store:
                return
            o0 = oc * P
            ow = min(P, O_ - o0)
            ob = opool.tile([P, TS], dt.bfloat16, tag="ob", name="ob")
            nc.vector.tensor_scalar_add(
                ob[:ow, :], ps[:ow, :], b_sb[:ow, oc : oc + 1]
            )
            nc.gpsimd.dma_start(
                out_d[o0 : o0 + ow, ts * TS : (ts + 1) * TS], ob[:ow, :]
            )

        def mm_unit(oc, wT, ts_list):
            pss = {}
            for ts in ts_list:
                pss[ts] = pspool.tile([P, TS], dt.float32, tag="ps", name="ps")
            for kc in range(NG_):
                for ts in ts_list:
                    nc.tensor.matmul(
                        pss[ts][:, :],
                        wT[:, kc, :],
                        xres[:, kc, ts * TS : (ts + 1) * TS],
                        start=(kc == 0),
                        stop=(kc == NG_ - 1),
                    )
            for ts in ts_list:
                store(oc, ts, pss[ts])

        def mm_unit_pair(oc_i, wT_i, oc_j, wT_j, ts):
            # two chunks' single-slice sweeps interleaved: consecutive
            # matmuls alternate PSUM banks (same-bank back-to-back costs
            # ~15ns/MM) and stationaries (LDWEIGHTS is hidden)
            ps_i = pspool.tile([P, TS], dt.float32, tag="ps", name="ps")
            ps_j = pspool.tile([P, TS], dt.float32, tag="ps", name="ps")
            for kc in range(NG_):
                for ps, wT in ((ps_i, wT_i), (ps_j, wT_j)):
                    nc.tensor.matmul(
                        ps[:, :],
                        wT[:, kc, :],
                        xres[:, kc, ts * TS : (ts + 1) * TS],
                        start=(kc == 0),
                        stop=(kc == NG_ - 1),
                    )
            store(oc_i, ts, ps_i)
            store(oc_j, ts, ps_j)

        wTs = dict(pre_wTs)

        def ensure(oc):
            if oc < n_oc and oc not in wTs:
                wTs[oc] = w_load(oc)

        ensure(0)
        ensure(1)
        if not hoist_x:
            x_load(0)
        ensure(2)
        if not hoist_x:
            x_load(1)
        ensure(3)
        if not hoist_x:
            for ts in range(2, n_ts):
                x_load(ts)
        for i in range(4, n_a):
            ensure(i)
        # A-units: token-slice 0 only, chunks interleaved in pairs
        for i in range(0, n_a - 1, 2):
            mm_unit_pair(i, wTs[i], i + 1, wTs[i + 1], 0)
        if n_a % 2:
            mm_unit(n_a - 1, wTs[n_a - 1], [0])
        # B-units; chunk i+w_bufs reuses wT slot i — load only after B-unit i
        rest = list(range(1, n_ts))
        if rest:
            for i in range(n_a):
                mm_unit(i, wTs[i], rest)
                ensure(n_a + i)
        # F-units; last chunk retires slices progressively ([0],[1],[2,3])
        # so the next iteration's x loads start staggered before the
        # iteration boundary instead of queueing after it
        for oc in range(n_a, n_oc):
            ensure(oc)
            if oc == n_oc - 1 and not hoist_x:
                mm_unit(oc, wTs[oc], [0])
                mm_unit(oc, wTs[oc], [1])
                mm_unit(oc, wTs[oc], list(range(2, n_ts)))
            else:
                mm_unit(oc, wTs[oc], list(range(n_ts)))
            ensure(oc + n_a)

    iters = cfg.get("iters", 1)
    if iters == 1:
        emit_iter()
    else:
        hints = (
            mybir.EngineType.PE,
            mybir.EngineType.DVE,
            mybir.EngineType.SP,
            mybir.EngineType.Activation,
            mybir.EngineType.Pool,
        )
        with tc.For_i(0, iters, 1, hint_engines=hints):
            emit_iter()


def build(t=T, in_=IN, o_pc=O_PC, iters=1, compile_=True, debug=False, **kw):
    cfg = dict(t=t, in_=in_, o_pc=o_pc, iters=iters, debug=debug, **kw)
    nc = bacc.Bacc("TRN2", target_bir_lowering=False, debug=False)
    with tile.TileContext(nc) as tc, ExitStack() as ctx:
        build_body(ctx, tc, cfg)
    if compile_:
        nc.compile()
    return nc


def make_in_maps(input, int_weight, scales, zeros, eq_scales, bias, n_cores=N_CORES):
    t = input.shape[0] * input.shape[1]
    in_ = input.shape[2]
    o_pc = int_weight.shape[0] // n_cores
    ng = in_ // GROUP
    n_oc = (o_pc + P_HOST - 1) // P_HOST
    o_pad = n_oc * P_HOST

    x2d = np.asarray(input, np.float32).reshape(t, in_) / np.asarray(
        eq_scales, np.float32
    )
    xt = (
        x2d.reshape(t, ng, P_HOST)
        .transpose(2, 1, 0)
        .astype(ml_dtypes.bfloat16, order="C")
        .reshape(P_HOST, ng * t)
    )
    wd = (
        (np.asarray(int_weight, np.float32) - np.asarray(zeros, np.float32))
        * np.asarray(scales, np.float32)
    ).reshape(int_weight.shape[0], in_)
    b1 = np.asarray(bias, np.float32)

    in_maps = []
    for c in range(n_cores):
        sl = slice(c * o_pc, (c + 1) * o_pc)
        wp = np.zeros((o_pad, in_), np.float32)
        wp[:o_pc] = wd[sl]
        wt = (
            wp.reshape(n_oc, P_HOST, ng, P_HOST)
            .transpose(0, 3, 2, 1)
            .astype(ml_dtypes.bfloat16, order="C")
            .reshape(n_oc * P_HOST, ng * P_HOST)
        )
        bpad = np.zeros(o_pad, np.float32)
        bpad[:o_pc] = b1[sl]
        bt = bpad.reshape(n_oc, P_HOST).T.copy()
        in_maps.append({"xt": xt, "wt": wt, "bt": np.ascontiguousarray(bt)})
    return in_maps


_NC_CACHE = {}


def kernel(input, int_weight, scales, zeros, eq_scales, bias):
    key = ("main", 1)
    if key not in _NC_CACHE:
        _NC_CACHE[key] = build()
    nc = _NC_CACHE[key]
    in_maps = make_in_maps(input, int_weight, scales, zeros, eq_scales, bias)
    run_bass_kernel_spmd(nc, in_maps, list(range(N_CORES)))
    res = run_bass_kernel_spmd(nc, in_maps, list(range(N_CORES)))
    outs = [np.asarray(res.results[c]["out"]) for c in range(N_CORES)]
    full = np.concatenate(outs, axis=0).astype(np.float32).T
    return np.ascontiguousarray(full).reshape(B, S, OUT)
